# revision 57
# baseline (speedup 1.0000x reference)
"""Trainium2 Bass kernel for multi-head attention (B=4, T=2048, C=1024, H=16).

Sharding: 8 cores = (batch b in 0..3) x (head-group g in 0..1, 8 heads each).
Per core: QKV projections for its 512 dims, attention for 8 heads, partial
output projection. Host sums the two per-batch partials and adds the biases
that fold out of the device computation:
  - bk drops entirely (softmax is invariant to per-query additive constants)
  - bv folds to host:   out += Wo @ bv   (softmax rows sum to 1)
  - bo added on host
  - bq is applied on-device on the Q-projection drain (DVE tensor_scalar);
    the 1/sqrt(dh) scale is folded into wq/bq on the host.

The kernel is paced by the scalar engine: softmax exp is 33.5M elements per
core at 1 elem/cycle/lane (~293us incl. per-instruction overhead), more than
all PE matmul work combined.  Schedule: K projection, Q projection for query
block 0, then attention group (qb0) starts immediately (priority-boosted) so
exp begins ~50us in; the V projection, remaining Q blocks, PV, normalize and
output projection all fill PE/DVE slack underneath the saturated ACT engine.

Device schedule (per core, emission order == program order):
  weights DMA; K proj (4 blocks); Q proj block 0; V proj (all, program-order
  before its PV consumers, de-prioritized below qb0 attention); for each
  query block: scores pairs (row-tiled, concurrent) -> exp on ACT ->
  ones-augmented P@V (row 64 = softmax denominator) -> reciprocal+broadcast
  normalize -> out-projection (bf16 out, summed on host).
"""
import numpy as np
import ml_dtypes

import concourse.bass as bass
import concourse.mybir as mybir
import concourse.tile as tile
from concourse import bacc

F32 = mybir.dt.float32
BF16 = mybir.dt.bfloat16
AF = mybir.ActivationFunctionType

B, T, C = 4, 2048, 1024
H, CH = 16, 64
G = 512            # dims per head-group (8 heads)
NCIN = 8           # 128-chunks of C
NCOUT = 4          # 128-chunks of G
NTB = 4            # 512-wide t blocks
NKC = 16           # 128-wide key chunks
NQB = 4            # 512-wide query blocks
SCALE = 1.0 / np.sqrt(CH)


def build_nc(debug=False):
    nc = bacc.Bacc()
    # Inputs are host-packed so every DMA moves 8KB-contiguous per-partition
    # lines (1KB lines run the single HWDGE ring at ~half throughput).
    xq = nc.declare_dram_parameter("xq", [128, NTB, NCIN, 512], BF16,
                                   isOutput=False)
    xk = nc.declare_dram_parameter("xk", [128, NTB, NCIN, 512], BF16,
                                   isOutput=False)
    xv = nc.declare_dram_parameter("xv", [128, NTB, NCIN, 512], BF16,
                                   isOutput=False)
    wq = nc.declare_dram_parameter("wq", [128, NCOUT, NCIN, 128], BF16,
                                   isOutput=False)
    wk = nc.declare_dram_parameter("wk", [128, NCOUT, NCIN, 128], BF16,
                                   isOutput=False)
    wv = nc.declare_dram_parameter("wv", [128, NCIN, G], BF16, isOutput=False)
    wo = nc.declare_dram_parameter("wo", [128, NCOUT, C], BF16,
                                   isOutput=False)
    bq = nc.declare_dram_parameter("bq", [128, NCOUT], F32, isOutput=False)
    out = nc.declare_dram_parameter("out", [16, 128, C], BF16, isOutput=True)

    with tile.TileContext(nc) as tc:
        with tc.tile_pool(name="persist", bufs=1) as persist, \
             tc.tile_pool(name="qtp", bufs=2) as qtp, \
             tc.tile_pool(name="xs", bufs=2) as xs, \
             tc.tile_pool(name="xvs", bufs=2) as xvs, \
             tc.tile_pool(name="eb", bufs=2) as eb, \
             tc.tile_pool(name="otp", bufs=2) as otp, \
             tc.tile_pool(name="dv", bufs=1) as dv, \
             tc.tile_pool(name="pp", bufs=2, space="PSUM") as pp, \
             tc.tile_pool(name="scp", bufs=2, space="PSUM") as scp, \
             tc.tile_pool(name="pvp", bufs=2, space="PSUM") as pvp:
            kt = [persist.tile([128, T], BF16, tag=f"kt{i}", name=f"kt{i}")
                  for i in range(NCOUT)]
            # V augmented: per-head column 64 is ones -> PV row 64 = denominator
            v_aug = persist.tile([128, NKC, 8, 65], BF16, tag="vaug")
            nc.vector.memset(v_aug[:, :, :, 64:65], 1.0)

            # Warm the PE clock (HAM un-throttles after ~3.4us of sustained
            # matmul activity) with junk matmuls while the input DMAs fly.
            wu = persist.tile([128, 128], BF16, tag="wu", name="wu")
            nc.vector.memset(wu[:, :], 0.0)
            wups = pp.tile([128, 512], F32, tag="proj", name="wups")
            for _ in range(48):
                nc.tensor.matmul(wups[:, 0:128], wu, wu, start=True,
                                 stop=True)

            # per-chunk weight tiles: whole-tile write tracking would
            # serialize the four co-slice DMAs if they shared one tile
            wk_sb = [persist.tile([128, NCIN, 128], BF16, tag=f"wk{co}",
                                  name=f"wk{co}") for co in range(NCOUT)]
            wq_sb = [persist.tile([128, NCIN, 128], BF16, tag=f"wq{co}",
                                  name=f"wq{co}") for co in range(NCOUT)]
            wv_sb = persist.tile([128, NCIN, G], BF16, tag="wv")
            wo_sb = persist.tile([128, NCOUT, C], BF16, tag="wo")
            bq_sb = persist.tile([128, NCOUT], F32, tag="bq")
            # qt tiles are pooled per (t-block, chunk); filled by Q projection
            # and read by that query block's scores.
            qt_tiles = {}

            # Lead-in DMAs split across the two HWDGE rings in need-order: the
            # K path on the Sync ring, the Q and V paths on the Activation
            # ring (idle until the first exp, so its triggers are free; only
            # dependency-free transfers go there -- a waiting trigger would
            # block the exps queued behind it).
            nc.sync.dma_start(out=wk_sb[0], in_=wk[:, 0])
            nc.scalar.dma_start(out=wq_sb[0], in_=wq[:, 0])

            def k_proj_cos(tb, xk_t, cos):
                for co in cos:
                    ps = pp.tile([128, 512], F32, tag="proj", name="psk")
                    for ci in range(NCIN):
                        nc.tensor.matmul(
                            ps, wk_sb[co][:, ci, :], xk_t[:, ci, :],
                            start=(ci == 0), stop=(ci == NCIN - 1))
                    nc.vector.tensor_copy(
                        out=kt[co][:, tb * 512:(tb + 1) * 512], in_=ps)

            def q_proj_cos(tb, xq_ci, cos):
                """xq_ci: callable ci -> ([128, 512] AP slice)."""
                for co in cos:
                    ps = pp.tile([128, 512], F32, tag="proj", name="psq")
                    for ci in range(NCIN):
                        nc.tensor.matmul(
                            ps, wq_sb[co][:, ci, :], xq_ci(ci),
                            start=(ci == 0), stop=(ci == NCIN - 1))
                    nc.vector.tensor_scalar_add(
                        qt_tiles[tb][co], ps, bq_sb[:, co:co + 1])

            # t-block 0 stream DMAs: the K path on the Sync ring; xq0 split
            # across both rings (half each) so the first Q chunk lands early.
            xk_t0 = xs.tile([128, NCIN, 512], BF16, tag="xs", name="xk_t")
            nc.sync.dma_start(out=xk_t0, in_=xk[:, 0, :, :])
            xq_t0a = persist.tile([128, 4, 512], BF16, tag="xq0a",
                                  name="xq_t0a")
            xq_t0b = xvs.tile([128, 4, 512], BF16, tag="xv", name="xq_t0b")
            nc.scalar.dma_start(out=xq_t0a, in_=xq[:, 0, 0:4, :])
            nc.scalar.dma_start(out=xq_t0b, in_=xq[:, 0, 4:8, :])
            nc.scalar.dma_start(out=bq_sb, in_=bq[:, :])
            qt_tiles[0] = [qtp.tile([128, 512], BF16, tag=f"qt{co}",
                                    name=f"qt0_{co}") for co in range(NCOUT)]
            for co in range(1, NCOUT):
                nc.sync.dma_start(out=wk_sb[co], in_=wk[:, co])
                nc.scalar.dma_start(out=wq_sb[co], in_=wq[:, co])
            # xq1 on the (otherwise idle) Activation ring: the interleaved
            # (1,0) group needs Q1.co0 by ~exp 32
            xq_late = {}
            xq_late[1] = xvs.tile([128, NCIN, 512], BF16, tag="xv",
                                  name="xq_t")
            nc.scalar.dma_start(out=xq_late[1], in_=xq[:, 1, :, :])
            nc.scalar.dma_start(out=wv_sb, in_=wv[:, :, :])
            xv_t0 = xvs.tile([128, NCIN, 512], BF16, tag="xv", name="xv_t")
            nc.scalar.dma_start(out=xv_t0, in_=xv[:, 0, :, :])

            def xq0_ci(ci):
                return (xq_t0a[:, ci, :] if ci < 4 else xq_t0b[:, ci - 4, :])

            # Only chunk co=0 of (K0, Q0) ahead of the attention priorities:
            # the first score group needs exactly that much.
            k_proj_cos(0, xk_t0, [0])
            q_proj_cos(0, xq0_ci, [0])

            # Front-load (priority BELOW the whole attention stream, which
            # preempts whenever its dependencies are met): remaining (K0, Q0)
            # chunks, K t-blocks 1-3, Q1 (needed by the interleaved qb1
            # groups), then the V projection, then Q2-3.
            pri_mark = tc.cur_priority
            tc.cur_priority += 50000

            # Only co=1 of (K0, Q0) here — co 2-3 are needed first by groups
            # (0,2)/(0,3) (~exp 64+) and are deferred past the V projection,
            # freeing ~7us of the PE window that bounds V completion.
            k_proj_cos(0, xk_t0, [1])
            q_proj_cos(0, xq0_ci, [1])
            xk_late = {}
            xk_late[1] = xs.tile([128, NCIN, 512], BF16, tag="xs",
                                 name="xk_t")
            nc.sync.dma_start(out=xk_late[1], in_=xk[:, 1, :, :])
            k_proj_cos(1, xk_late[1], [0, 1, 2, 3])
            # Q1 chunk 0 right here: the interleaved (1,0) group needs only
            # this much by ~exp 32; the rest of Q1 follows K2/K3.
            qt_tiles[1] = [qtp.tile([128, 512], BF16, tag=f"qt{co}",
                                    name=f"qt1_{co}") for co in range(NCOUT)]
            q_proj_cos(1, lambda ci: xq_late[1][:, ci, :], [0])
            for tb in (2, 3):
                xk_late[tb] = xs.tile([128, NCIN, 512], BF16, tag="xs",
                                      name="xk_t")
                nc.sync.dma_start(out=xk_late[tb], in_=xk[:, tb, :, :])
                k_proj_cos(tb, xk_late[tb], [0, 1, 2, 3])
            q_proj_cos(1, lambda ci: xq_late[1][:, ci, :], [1, 2, 3])

            # ---------- V projection ----------
            nc.sync.dma_start(out=wo_sb, in_=wo[:, :, :])
            for tb in range(NTB):
                if tb == 0:
                    xv_t = xv_t0
                else:
                    xv_t = xvs.tile([128, NCIN, 512], BF16, tag="xv",
                                    name="xv_t")
                    nc.sync.dma_start(out=xv_t, in_=xv[:, tb, :, :])
                for sub in range(4):
                    tcix = tb * 4 + sub
                    ps = pp.tile([128, 512], F32, tag="proj", name="psv")
                    for ci in range(NCIN):
                        nc.tensor.matmul(
                            ps, xv_t[:, ci, sub * 128:(sub + 1) * 128],
                            wv_sb[:, ci, :],
                            start=(ci == 0), stop=(ci == NCIN - 1))
                    nc.vector.tensor_copy(out=v_aug[:, tcix, :, 0:64], in_=ps)

            # Deferred (K0, Q0) co 2-3, with the t-block-0 inputs re-DMA'd
            # into fresh pool slots (reusing xk_t0/xq_t0b here would extend
            # their lifetimes and stall the stream-pool rotations above).
            xk_t0b = xs.tile([128, NCIN, 512], BF16, tag="xs", name="xk_t0b")
            nc.sync.dma_start(out=xk_t0b, in_=xk[:, 0, :, :])
            xq_t0c = xvs.tile([128, 4, 512], BF16, tag="xv", name="xq_t0c")
            nc.sync.dma_start(out=xq_t0c, in_=xq[:, 0, 4:8, :])
            k_proj_cos(0, xk_t0b, [2, 3])
            q_proj_cos(0, lambda ci: (xq_t0a[:, ci, :] if ci < 4
                                      else xq_t0c[:, ci - 4, :]), [2, 3])

            for tb in (2, 3):
                xq_late[tb] = xvs.tile([128, NCIN, 512], BF16, tag="xv",
                                       name="xq_t")
                nc.sync.dma_start(out=xq_late[tb], in_=xq[:, tb, :, :])

            out_dma_count = [0]

            ot_tiles = {}

            def attention_group(qb, p):
                qt = qt_tiles[qb]
                if qb not in ot_tiles:
                    ot_tiles[qb] = {}
                ot_tiles[qb][p] = otp.tile([128, 512], BF16, tag=f"ot{p}",
                                           name=f"ot{p}")
                if True:
                    # E split in quarter-group tiles (same 80KB total): each
                    # quarter's recycle-wait then tracks the V projection's
                    # incremental t-chunk progress (PV consumes kc as the
                    # matching V chunk lands) instead of half/full V.
                    eh = [eb.tile([128, NKC // 4, 2, 512], BF16,
                                  tag=f"e{j}", bufs=(3 if j < 2 else 2),
                                  name=f"e{j}") for j in range(4)]
                    for kc in range(NKC):
                        e01 = eh[kc // 4][:, kc % 4, :, :]
                        psc = scp.tile([128, 2, 512], F32, tag="sc",
                                       name="psc")
                        ksl = slice(kc * 128, (kc + 1) * 128)
                        nc.tensor.matmul(
                            psc[:, 0, :], kt[p][0:64, ksl],
                            qt[p][0:64, :], start=True, stop=True)
                        nc.tensor.matmul(
                            psc[:, 1, :], kt[p][64:128, ksl],
                            qt[p][64:128, :], start=True, stop=True)
                        nc.scalar.activation(e01, psc, AF.Exp)
                    # P @ V with ones-augmented V: row 64 = denominator
                    pv0 = pvp.tile([128, 512], F32, tag="pv", name="pv0")
                    pv1 = pvp.tile([128, 512], F32, tag="pv", name="pv1")
                    for kc in range(NKC):
                        e01 = eh[kc // 4][:, kc % 4, :, :]
                        nc.tensor.matmul(
                            pv0[0:65, :], v_aug[:, kc, 2 * p, :],
                            e01[:, 0, :],
                            start=(kc == 0), stop=(kc == NKC - 1))
                        nc.tensor.matmul(
                            pv1[0:65, :], v_aug[:, kc, 2 * p + 1, :],
                            e01[:, 1, :],
                            start=(kc == 0), stop=(kc == NKC - 1))
                    d_sb = dv.tile([1, 2, 512], F32, tag="dsb", name="d_sb")
                    nc.vector.tensor_copy(out=d_sb[0:1, 0, :],
                                          in_=pv0[64:65, :])
                    nc.vector.tensor_copy(out=d_sb[0:1, 1, :],
                                          in_=pv1[64:65, :])
                    nc.vector.reciprocal_approx_fast(d_sb[0:1, :, :],
                                                     d_sb[0:1, :, :])
                    dbc_lo = dv.tile([64, 512], F32, tag="b0", name="dbc_lo")
                    dbc_hi = dv.tile([64, 512], F32, tag="b1", name="dbc_hi")
                    nc.gpsimd.partition_broadcast(dbc_lo[:, :],
                                                  d_sb[0:1, 0, :],
                                                  channels=64)
                    nc.gpsimd.partition_broadcast(dbc_hi[:, :],
                                                  d_sb[0:1, 1, :],
                                                  channels=64)
                    nc.vector.tensor_mul(ot_tiles[qb][p][0:64, :],
                                         pv0[0:64, :], dbc_lo[:, :])
                    nc.vector.tensor_mul(ot_tiles[qb][p][64:128, :],
                                         pv1[0:64, :], dbc_hi[:, :])

            def outproj_qb(qb):
                ot_p = ot_tiles[qb]
                for tcx in range(4):
                    oj = dv.tile([128, 2, 512], BF16, tag="oj", bufs=1,
                                 name="oj")
                    for n in range(2):
                        pj = pp.tile([128, 512], F32, tag="proj", name="pj")
                        for p in range(NCOUT):
                            nc.tensor.matmul(
                                pj, ot_p[p][:, tcx * 128:(tcx + 1) * 128],
                                wo_sb[:, p, n * 512:(n + 1) * 512],
                                start=(p == 0), stop=(p == NCOUT - 1))
                        nc.vector.tensor_copy(out=oj[:, n, :], in_=pj)
                    # Deprioritize the output-DMA trigger far below every
                    # input trigger: the Sync sequencer executes its queue in
                    # (static, priority-derived) order, and a high-priority
                    # out trigger waiting on oj would head-of-line block the
                    # xv/xq input DMAs queued behind it.
                    pri_save = tc.cur_priority
                    tc.cur_priority = 200000 + out_dma_count[0]
                    out_dma_count[0] += 1
                    nc.sync.dma_start(out=out[qb * 4 + tcx, :, :], in_=oj)
                    tc.cur_priority = pri_save

            for tb in (2, 3):
                qt_tiles[tb] = [
                    qtp.tile([128, 512], BF16, tag=f"qt{co}",
                             name=f"qt{tb}_{co}") for co in range(NCOUT)]
                q_proj_cos(tb, (lambda t: lambda ci: xq_late[t][:, ci, :])(tb),
                           [0, 1, 2, 3])

            # All attention goes into the reserved priority slot right after
            # the (K0, Q0) projections.  The first two query blocks are
            # group-interleaved: it gives the exp stream two extra groups of
            # runway before it hits the V-projection-gated E-buffer recycle.
            with tc.high_priority(offset=tc.cur_priority - pri_mark):
                for qb, p in [(0, 0), (0, 1), (1, 0), (1, 1),
                              (0, 2), (0, 3), (1, 2), (1, 3)]:
                    attention_group(qb, p)
                    if (qb, p) == (0, 3):
                        outproj_qb(0)
                    elif (qb, p) == (1, 3):
                        outproj_qb(1)
                for qb in (2, 3):
                    for p in range(NCOUT):
                        attention_group(qb, p)
                    outproj_qb(qb)
    nc.finalize()
    return nc


_CACHE = {}


def _get_runner():
    """Compile once per process; return f(in_maps) -> list of out dicts."""
    if "runner" in _CACHE:
        return _CACHE["runner"]
    import jax
    from jax.sharding import Mesh, PartitionSpec
    from jax.experimental.shard_map import shard_map
    from concourse import bass2jax

    nc = build_nc()
    bass2jax.install_neuronx_cc_hook()
    in_names, out_names, out_avals, zero_shapes = [], [], [], []
    for alloc in nc.m.functions[0].allocations:
        if not isinstance(alloc, mybir.MemoryLocationSet):
            continue
        name = alloc.memorylocations[0].name
        if alloc.kind == "ExternalInput":
            if name != "partition_id":
                in_names.append(name)
        elif alloc.kind == "ExternalOutput":
            out_names.append(name)
            shape = tuple(alloc.tensor_shape)
            dtype = mybir.dt.np(alloc.dtype)
            out_avals.append(jax.core.ShapedArray(shape, dtype))
            zero_shapes.append((shape, dtype))
    n_params = len(in_names)
    all_names = tuple(in_names + out_names)
    donate = tuple(range(n_params, n_params + len(out_names)))
    has_pid = nc.partition_id_tensor is not None

    def _body(*args):
        operands = list(args)
        names = all_names
        if has_pid:
            operands.append(bass2jax.partition_id_tensor())
            names = all_names + ("partition_id",)
        outs = bass2jax._bass_exec_p.bind(
            *operands, out_avals=tuple(out_avals), in_names=names,
            out_names=tuple(out_names), lowering_input_output_aliases=(),
            sim_require_finite=False, sim_require_nnan=False, nc=nc)
        return tuple(outs)

    devices = jax.devices()[:8]
    mesh = Mesh(np.asarray(devices), ("core",))
    specs = (PartitionSpec("core"),) * (n_params + len(out_names))
    f = jax.jit(shard_map(_body, mesh=mesh, in_specs=specs,
                          out_specs=(PartitionSpec("core"),) * len(out_names),
                          check_rep=False),
                donate_argnums=donate, keep_unused=True)

    def run(in_maps):
        concat_in = [np.concatenate([m[n] for m in in_maps], axis=0)
                     for n in in_names]
        concat_zeros = [np.zeros((8 * s[0], *s[1:]), d) for s, d in zero_shapes]
        outs = f(*concat_in, *concat_zeros)
        res = []
        for c in range(8):
            res.append({name: np.asarray(outs[i]).reshape(8, *out_avals[i].shape)[c]
                        for i, name in enumerate(out_names)})
        return res

    _CACHE["runner"] = run
    _CACHE["nc"] = nc
    return run


def _pack_x(x2d):
    """[C, T] -> [128, NTB, NCIN, 512] bf16 (8KB-contiguous partition lines)."""
    return np.ascontiguousarray(
        x2d.reshape(NCIN, 128, NTB, 512).transpose(1, 2, 0, 3)).astype(
            ml_dtypes.bfloat16)


def _pack_w(wT, nco):
    """[C_in, C_out] -> [128, nco, C_out] bf16."""
    return np.ascontiguousarray(
        wT.reshape(nco, 128, wT.shape[1]).transpose(1, 0, 2)).astype(
            ml_dtypes.bfloat16)


def _pack_w4(wT):
    """[C, G] -> [128, NCOUT, NCIN, 128] bf16 (co-major for split DMA)."""
    return np.ascontiguousarray(
        wT.reshape(NCIN, 128, NCOUT, 128).transpose(1, 2, 0, 3)).astype(
            ml_dtypes.bfloat16)


def make_in_maps(k, q, v, Wk, bk, Wq, bq, Wv, bv, Wo, bo):
    in_maps = []
    for c in range(8):
        b, g = divmod(c, 2)
        gs, ge = g * G, (g + 1) * G
        bqs = (bq[gs:ge] * SCALE).reshape(NCOUT, 128).T
        in_maps.append({
            "xq": _pack_x(q[b].T),
            "xk": _pack_x(k[b].T),
            "xv": _pack_x(v[b].T),
            "wq": _pack_w4(Wq[gs:ge, :].T * SCALE),
            "wk": _pack_w4(Wk[gs:ge, :].T),
            "wv": _pack_w(Wv[gs:ge, :].T, NCIN),
            "wo": _pack_w(Wo[:, gs:ge].T, NCOUT),
            "bq": np.ascontiguousarray(bqs, dtype=np.float32),
        })
    return in_maps


def kernel(k, q, v, Wk, bk, Wq, bq, Wv, bv, Wo, bo):
    k = np.asarray(k, dtype=np.float32)
    q = np.asarray(q, dtype=np.float32)
    v = np.asarray(v, dtype=np.float32)
    Wk, bk = np.asarray(Wk, np.float32), np.asarray(bk, np.float32)
    Wq, bq = np.asarray(Wq, np.float32), np.asarray(bq, np.float32)
    Wv, bv = np.asarray(Wv, np.float32), np.asarray(bv, np.float32)
    Wo, bo = np.asarray(Wo, np.float32), np.asarray(bo, np.float32)

    in_maps = make_in_maps(k, q, v, Wk, bk, Wq, bq, Wv, bv, Wo, bo)
    run = _get_runner()
    res = run(in_maps)
    host_bias = (bo + Wo @ bv).astype(np.float32)
    out = np.empty((B, T, C), np.float32)
    for b in range(B):
        out[b] = (res[2 * b]["out"].astype(np.float32).reshape(T, C)
                  + res[2 * b + 1]["out"].astype(np.float32).reshape(T, C)
                  + host_bias[None, :])
    return out


# revision 60
# speedup vs baseline: 1.1988x; 1.1988x over previous
"""Trainium2 Bass kernel for multi-head attention (B=4, T=2048, C=1024, H=16).

Sharding: 8 cores = (batch b in 0..3) x (head-group g in 0..1, 8 heads each).
Per core: QKV projections for its 512 dims, attention for 8 heads, partial
output projection. Host sums the two per-batch partials and adds the biases
that fold out of the device computation:
  - bk drops entirely (softmax is invariant to per-query additive constants)
  - bv folds to host:   out += Wo @ bv   (softmax rows sum to 1)
  - bo added on host
  - bq is applied on-device on the Q-projection drain (DVE tensor_scalar);
    the 1/sqrt(dh) scale is folded into wq/bq on the host.

The kernel is paced by the scalar engine: softmax exp is 33.5M elements per
core at 1 elem/cycle/lane (~293us incl. per-instruction overhead), more than
all PE matmul work combined.  Schedule: K projection, Q projection for query
block 0, then attention group (qb0) starts immediately (priority-boosted) so
exp begins ~50us in; the V projection, remaining Q blocks, PV, normalize and
output projection all fill PE/DVE slack underneath the saturated ACT engine.

Device schedule (per core, emission order == program order):
  weights DMA; K proj (4 blocks); Q proj block 0; V proj (all, program-order
  before its PV consumers, de-prioritized below qb0 attention); for each
  query block: scores pairs (row-tiled, concurrent) -> exp on ACT ->
  ones-augmented P@V (row 64 = softmax denominator) -> reciprocal+broadcast
  normalize -> out-projection (bf16 out, summed on host).
"""
import numpy as np
import ml_dtypes

import concourse.bass as bass
import concourse.mybir as mybir
import concourse.tile as tile
from concourse import bacc

F32 = mybir.dt.float32
BF16 = mybir.dt.bfloat16
AF = mybir.ActivationFunctionType

B, T, C = 4, 2048, 1024
H, CH = 16, 64
G = 512            # dims per head-group (8 heads)
NCIN = 8           # 128-chunks of C
NCOUT = 4          # 128-chunks of G
NTB = 4            # 512-wide t blocks
NKC = 16           # 128-wide key chunks
NQB = 4            # 512-wide query blocks
SCALE = 1.0 / np.sqrt(CH)


def build_nc(debug=False):
    nc = bacc.Bacc()
    # Inputs are host-packed so every DMA moves 8KB-contiguous per-partition
    # lines (1KB lines run the single HWDGE ring at ~half throughput).
    xq = nc.declare_dram_parameter("xq", [128, NTB, NCIN, 512], BF16,
                                   isOutput=False)
    xk = nc.declare_dram_parameter("xk", [128, NTB, NCIN, 512], BF16,
                                   isOutput=False)
    xv = nc.declare_dram_parameter("xv", [128, NTB, NCIN, 512], BF16,
                                   isOutput=False)
    wq = nc.declare_dram_parameter("wq", [128, NCOUT, NCIN, 128], BF16,
                                   isOutput=False)
    wk = nc.declare_dram_parameter("wk", [128, NCOUT, NCIN, 128], BF16,
                                   isOutput=False)
    wv = nc.declare_dram_parameter("wv", [128, NCIN, G], BF16, isOutput=False)
    wo = nc.declare_dram_parameter("wo", [128, NCOUT, C], BF16,
                                   isOutput=False)
    bq = nc.declare_dram_parameter("bq", [128, NCOUT], F32, isOutput=False)
    out = nc.declare_dram_parameter("out", [16, 128, C], BF16, isOutput=True)

    with tile.TileContext(nc) as tc:
        with tc.tile_pool(name="persist", bufs=1) as persist, \
             tc.tile_pool(name="qtp", bufs=2) as qtp, \
             tc.tile_pool(name="xs", bufs=2) as xs, \
             tc.tile_pool(name="xvs", bufs=2) as xvs, \
             tc.tile_pool(name="eb", bufs=2) as eb, \
             tc.tile_pool(name="otp", bufs=2) as otp, \
             tc.tile_pool(name="dv", bufs=1) as dv, \
             tc.tile_pool(name="pp", bufs=2, space="PSUM") as pp, \
             tc.tile_pool(name="scp", bufs=2, space="PSUM") as scp, \
             tc.tile_pool(name="pvp", bufs=2, space="PSUM") as pvp:
            kt = [persist.tile([128, T], BF16, tag=f"kt{i}", name=f"kt{i}")
                  for i in range(NCOUT)]
            # V augmented: per-head column 64 is ones -> PV row 64 = denominator
            v_aug = persist.tile([128, NKC, 8, 65], BF16, tag="vaug")
            nc.vector.memset(v_aug[:, :, :, 64:65], 1.0)

            # Warm the PE clock (HAM un-throttles after ~3.4us of sustained
            # matmul activity) with junk matmuls while the input DMAs fly.
            wu = persist.tile([128, 128], BF16, tag="wu", name="wu")
            nc.vector.memset(wu[:, :], 0.0)
            wups = pp.tile([128, 512], F32, tag="proj", name="wups")
            for _ in range(48):
                nc.tensor.matmul(wups[:, 0:128], wu, wu, start=True,
                                 stop=True)

            # per-chunk weight tiles: whole-tile write tracking would
            # serialize the four co-slice DMAs if they shared one tile
            wk_sb = [persist.tile([128, NCIN, 128], BF16, tag=f"wk{co}",
                                  name=f"wk{co}") for co in range(NCOUT)]
            wq_sb = [persist.tile([128, NCIN, 128], BF16, tag=f"wq{co}",
                                  name=f"wq{co}") for co in range(NCOUT)]
            wv_sb = persist.tile([128, NCIN, G], BF16, tag="wv")
            wo_sb = persist.tile([128, NCOUT, C], BF16, tag="wo")
            bq_sb = persist.tile([128, NCOUT], F32, tag="bq")
            # qt tiles are pooled per (t-block, chunk); filled by Q projection
            # and read by that query block's scores.
            qt_tiles = {}

            # Lead-in DMAs split across the two HWDGE rings in need-order: the
            # K path on the Sync ring, the Q and V paths on the Activation
            # ring (idle until the first exp, so its triggers are free; only
            # dependency-free transfers go there -- a waiting trigger would
            # block the exps queued behind it).
            nc.sync.dma_start(out=wk_sb[0], in_=wk[:, 0])
            nc.scalar.dma_start(out=wq_sb[0], in_=wq[:, 0])

            def k_proj_cos(tb, xk_t, cos):
                for co in cos:
                    ps = pp.tile([128, 512], F32, tag="proj", name="psk")
                    for ci in range(NCIN):
                        nc.tensor.matmul(
                            ps, wk_sb[co][:, ci, :], xk_t[:, ci, :],
                            start=(ci == 0), stop=(ci == NCIN - 1))
                    nc.vector.tensor_copy(
                        out=kt[co][:, tb * 512:(tb + 1) * 512], in_=ps)

            def q_proj_cos(tb, xq_ci, cos):
                """xq_ci: callable ci -> ([128, 512] AP slice)."""
                for co in cos:
                    ps = pp.tile([128, 512], F32, tag="proj", name="psq")
                    for ci in range(NCIN):
                        nc.tensor.matmul(
                            ps, wq_sb[co][:, ci, :], xq_ci(ci),
                            start=(ci == 0), stop=(ci == NCIN - 1))
                    nc.vector.tensor_scalar_add(
                        qt_tiles[tb][co], ps, bq_sb[:, co:co + 1])

            # t-block 0 stream DMAs: the K path on the Sync ring; xq0 split
            # across both rings (half each) so the first Q chunk lands early.
            xk_t0 = xs.tile([128, NCIN, 512], BF16, tag="xs", name="xk_t")
            nc.sync.dma_start(out=xk_t0, in_=xk[:, 0, :, :])
            xq_t0a = persist.tile([128, 4, 512], BF16, tag="xq0a",
                                  name="xq_t0a")
            xq_t0b = xvs.tile([128, 4, 512], BF16, tag="xv", name="xq_t0b")
            nc.scalar.dma_start(out=xq_t0a, in_=xq[:, 0, 0:4, :])
            nc.scalar.dma_start(out=xq_t0b, in_=xq[:, 0, 4:8, :])
            nc.scalar.dma_start(out=bq_sb, in_=bq[:, :])
            qt_tiles[0] = [qtp.tile([128, 512], BF16, tag=f"qt{co}",
                                    name=f"qt0_{co}") for co in range(NCOUT)]
            for co in range(1, NCOUT):
                nc.sync.dma_start(out=wk_sb[co], in_=wk[:, co])
                nc.scalar.dma_start(out=wq_sb[co], in_=wq[:, co])
            # xq1 on the (otherwise idle) Activation ring: the interleaved
            # (1,0) group needs Q1.co0 by ~exp 32
            xq_late = {}
            xq_late[1] = xvs.tile([128, NCIN, 512], BF16, tag="xv",
                                  name="xq_t")
            nc.scalar.dma_start(out=xq_late[1], in_=xq[:, 1, :, :])
            nc.scalar.dma_start(out=wv_sb, in_=wv[:, :, :])
            xv_t0 = xvs.tile([128, NCIN, 512], BF16, tag="xv", name="xv_t")
            nc.scalar.dma_start(out=xv_t0, in_=xv[:, 0, :, :])

            def xq0_ci(ci):
                return (xq_t0a[:, ci, :] if ci < 4 else xq_t0b[:, ci - 4, :])

            # Only chunk co=0 of (K0, Q0) ahead of the attention priorities:
            # the first score group needs exactly that much.
            k_proj_cos(0, xk_t0, [0])
            q_proj_cos(0, xq0_ci, [0])

            # Front-load (priority BELOW the whole attention stream, which
            # preempts whenever its dependencies are met): remaining (K0, Q0)
            # chunks, K t-blocks 1-3, Q1 (needed by the interleaved qb1
            # groups), then the V projection, then Q2-3.
            pri_mark = tc.cur_priority
            tc.cur_priority += 50000

            # Only co=1 of (K0, Q0) here — co 2-3 are needed first by groups
            # (0,2)/(0,3) (~exp 64+) and are deferred past the V projection,
            # freeing ~7us of the PE window that bounds V completion.
            k_proj_cos(0, xk_t0, [1])
            q_proj_cos(0, xq0_ci, [1])
            xk_late = {}
            xk_late[1] = xs.tile([128, NCIN, 512], BF16, tag="xs",
                                 name="xk_t")
            nc.sync.dma_start(out=xk_late[1], in_=xk[:, 1, :, :])
            k_proj_cos(1, xk_late[1], [0, 1])
            # Q1 chunk 0 right here: the interleaved (1,0) group needs only
            # this much by ~exp 32; the rest of Q1 follows K2/K3.
            qt_tiles[1] = [qtp.tile([128, 512], BF16, tag=f"qt{co}",
                                    name=f"qt1_{co}") for co in range(NCOUT)]
            q_proj_cos(1, lambda ci: xq_late[1][:, ci, :], [0])
            for tb in (2, 3):
                xk_late[tb] = xs.tile([128, NCIN, 512], BF16, tag="xs",
                                      name="xk_t")
                nc.sync.dma_start(out=xk_late[tb], in_=xk[:, tb, :, :])
                k_proj_cos(tb, xk_late[tb], [0, 1])
            q_proj_cos(1, lambda ci: xq_late[1][:, ci, :], [1, 2, 3])

            # ---------- V projection ----------
            nc.sync.dma_start(out=wo_sb, in_=wo[:, :, :])
            for tb in range(NTB):
                if tb == 0:
                    xv_t = xv_t0
                else:
                    xv_t = xvs.tile([128, NCIN, 512], BF16, tag="xv",
                                    name="xv_t")
                    nc.sync.dma_start(out=xv_t, in_=xv[:, tb, :, :])
                for sub in range(4):
                    tcix = tb * 4 + sub
                    ps = pp.tile([128, 512], F32, tag="proj", name="psv")
                    for ci in range(NCIN):
                        nc.tensor.matmul(
                            ps, xv_t[:, ci, sub * 128:(sub + 1) * 128],
                            wv_sb[:, ci, :],
                            start=(ci == 0), stop=(ci == NCIN - 1))
                    nc.vector.tensor_copy(out=v_aug[:, tcix, :, 0:64], in_=ps)

            # Deferred (K0, Q0) co 2-3, with the t-block-0 inputs re-DMA'd
            # into fresh pool slots (reusing xk_t0/xq_t0b here would extend
            # their lifetimes and stall the stream-pool rotations above).
            for tb in range(NTB):
                xk_b = xs.tile([128, NCIN, 512], BF16, tag="xs",
                               name="xk_tb")
                nc.sync.dma_start(out=xk_b, in_=xk[:, tb, :, :])
                k_proj_cos(tb, xk_b, [2, 3])
            xq_t0c = xvs.tile([128, 4, 512], BF16, tag="xv", name="xq_t0c")
            nc.sync.dma_start(out=xq_t0c, in_=xq[:, 0, 4:8, :])
            q_proj_cos(0, lambda ci: (xq_t0a[:, ci, :] if ci < 4
                                      else xq_t0c[:, ci - 4, :]), [2, 3])

            for tb in (2, 3):
                xq_late[tb] = xvs.tile([128, NCIN, 512], BF16, tag="xv",
                                       name="xq_t")
                nc.sync.dma_start(out=xq_late[tb], in_=xq[:, tb, :, :])

            out_dma_count = [0]

            ot_tiles = {}

            def attention_group(qb, p):
                qt = qt_tiles[qb]
                if qb not in ot_tiles:
                    ot_tiles[qb] = {}
                ot_tiles[qb][p] = otp.tile([128, 512], BF16, tag=f"ot{p}",
                                           name=f"ot{p}")
                if True:
                    # E split in quarter-group tiles (same 80KB total): each
                    # quarter's recycle-wait then tracks the V projection's
                    # incremental t-chunk progress (PV consumes kc as the
                    # matching V chunk lands) instead of half/full V.
                    eh = [eb.tile([128, NKC // 4, 2, 512], BF16,
                                  tag=f"e{j}", bufs=(3 if j < 2 else 2),
                                  name=f"e{j}") for j in range(4)]
                    for kc in range(NKC):
                        e01 = eh[kc // 4][:, kc % 4, :, :]
                        psc = scp.tile([128, 2, 512], F32, tag="sc",
                                       name="psc")
                        ksl = slice(kc * 128, (kc + 1) * 128)
                        nc.tensor.matmul(
                            psc[:, 0, :], kt[p][0:64, ksl],
                            qt[p][0:64, :], start=True, stop=True)
                        nc.tensor.matmul(
                            psc[:, 1, :], kt[p][64:128, ksl],
                            qt[p][64:128, :], start=True, stop=True)
                        nc.scalar.activation(e01, psc, AF.Exp)
                    # P @ V with ones-augmented V: row 64 = denominator
                    pv0 = pvp.tile([128, 512], F32, tag="pv", name="pv0")
                    pv1 = pvp.tile([128, 512], F32, tag="pv", name="pv1")
                    for kc in range(NKC):
                        e01 = eh[kc // 4][:, kc % 4, :, :]
                        nc.tensor.matmul(
                            pv0[0:65, :], v_aug[:, kc, 2 * p, :],
                            e01[:, 0, :],
                            start=(kc == 0), stop=(kc == NKC - 1))
                        nc.tensor.matmul(
                            pv1[0:65, :], v_aug[:, kc, 2 * p + 1, :],
                            e01[:, 1, :],
                            start=(kc == 0), stop=(kc == NKC - 1))
                    d_sb = dv.tile([1, 2, 512], F32, tag="dsb", name="d_sb")
                    nc.vector.tensor_copy(out=d_sb[0:1, 0, :],
                                          in_=pv0[64:65, :])
                    nc.vector.tensor_copy(out=d_sb[0:1, 1, :],
                                          in_=pv1[64:65, :])
                    nc.vector.reciprocal_approx_fast(d_sb[0:1, :, :],
                                                     d_sb[0:1, :, :])
                    dbc_lo = dv.tile([64, 512], F32, tag="b0", name="dbc_lo")
                    dbc_hi = dv.tile([64, 512], F32, tag="b1", name="dbc_hi")
                    nc.gpsimd.partition_broadcast(dbc_lo[:, :],
                                                  d_sb[0:1, 0, :],
                                                  channels=64)
                    nc.gpsimd.partition_broadcast(dbc_hi[:, :],
                                                  d_sb[0:1, 1, :],
                                                  channels=64)
                    nc.vector.tensor_mul(ot_tiles[qb][p][0:64, :],
                                         pv0[0:64, :], dbc_lo[:, :])
                    nc.vector.tensor_mul(ot_tiles[qb][p][64:128, :],
                                         pv1[0:64, :], dbc_hi[:, :])

            def outproj_qb(qb):
                ot_p = ot_tiles[qb]
                for tcx in range(4):
                    oj = dv.tile([128, 2, 512], BF16, tag="oj", bufs=1,
                                 name="oj")
                    for n in range(2):
                        pj = pp.tile([128, 512], F32, tag="proj", name="pj")
                        for p in range(NCOUT):
                            nc.tensor.matmul(
                                pj, ot_p[p][:, tcx * 128:(tcx + 1) * 128],
                                wo_sb[:, p, n * 512:(n + 1) * 512],
                                start=(p == 0), stop=(p == NCOUT - 1))
                        nc.vector.tensor_copy(out=oj[:, n, :], in_=pj)
                    # Deprioritize the output-DMA trigger far below every
                    # input trigger: the Sync sequencer executes its queue in
                    # (static, priority-derived) order, and a high-priority
                    # out trigger waiting on oj would head-of-line block the
                    # xv/xq input DMAs queued behind it.
                    pri_save = tc.cur_priority
                    tc.cur_priority = 200000 + out_dma_count[0]
                    out_dma_count[0] += 1
                    nc.sync.dma_start(out=out[qb * 4 + tcx, :, :], in_=oj)
                    tc.cur_priority = pri_save

            for tb in (2, 3):
                qt_tiles[tb] = [
                    qtp.tile([128, 512], BF16, tag=f"qt{co}",
                             name=f"qt{tb}_{co}") for co in range(NCOUT)]
                q_proj_cos(tb, (lambda t: lambda ci: xq_late[t][:, ci, :])(tb),
                           [0, 1, 2, 3])

            # All attention goes into the reserved priority slot right after
            # the (K0, Q0) projections.  The first two query blocks are
            # group-interleaved: it gives the exp stream two extra groups of
            # runway before it hits the V-projection-gated E-buffer recycle.
            with tc.high_priority(offset=tc.cur_priority - pri_mark):
                for qb, p in [(0, 0), (0, 1), (1, 0), (1, 1),
                              (0, 2), (0, 3), (1, 2), (1, 3)]:
                    attention_group(qb, p)
                    if (qb, p) == (0, 3):
                        outproj_qb(0)
                    elif (qb, p) == (1, 3):
                        outproj_qb(1)
                for qb in (2, 3):
                    for p in range(NCOUT):
                        attention_group(qb, p)
                    outproj_qb(qb)
    nc.finalize()
    return nc


_CACHE = {}


def _get_runner():
    """Compile once per process; return f(in_maps) -> list of out dicts."""
    if "runner" in _CACHE:
        return _CACHE["runner"]
    import jax
    from jax.sharding import Mesh, PartitionSpec
    from jax.experimental.shard_map import shard_map
    from concourse import bass2jax

    nc = build_nc()
    bass2jax.install_neuronx_cc_hook()
    in_names, out_names, out_avals, zero_shapes = [], [], [], []
    for alloc in nc.m.functions[0].allocations:
        if not isinstance(alloc, mybir.MemoryLocationSet):
            continue
        name = alloc.memorylocations[0].name
        if alloc.kind == "ExternalInput":
            if name != "partition_id":
                in_names.append(name)
        elif alloc.kind == "ExternalOutput":
            out_names.append(name)
            shape = tuple(alloc.tensor_shape)
            dtype = mybir.dt.np(alloc.dtype)
            out_avals.append(jax.core.ShapedArray(shape, dtype))
            zero_shapes.append((shape, dtype))
    n_params = len(in_names)
    all_names = tuple(in_names + out_names)
    donate = tuple(range(n_params, n_params + len(out_names)))
    has_pid = nc.partition_id_tensor is not None

    def _body(*args):
        operands = list(args)
        names = all_names
        if has_pid:
            operands.append(bass2jax.partition_id_tensor())
            names = all_names + ("partition_id",)
        outs = bass2jax._bass_exec_p.bind(
            *operands, out_avals=tuple(out_avals), in_names=names,
            out_names=tuple(out_names), lowering_input_output_aliases=(),
            sim_require_finite=False, sim_require_nnan=False, nc=nc)
        return tuple(outs)

    devices = jax.devices()[:8]
    mesh = Mesh(np.asarray(devices), ("core",))
    specs = (PartitionSpec("core"),) * (n_params + len(out_names))
    f = jax.jit(shard_map(_body, mesh=mesh, in_specs=specs,
                          out_specs=(PartitionSpec("core"),) * len(out_names),
                          check_rep=False),
                donate_argnums=donate, keep_unused=True)

    def run(in_maps):
        concat_in = [np.concatenate([m[n] for m in in_maps], axis=0)
                     for n in in_names]
        concat_zeros = [np.zeros((8 * s[0], *s[1:]), d) for s, d in zero_shapes]
        outs = f(*concat_in, *concat_zeros)
        res = []
        for c in range(8):
            res.append({name: np.asarray(outs[i]).reshape(8, *out_avals[i].shape)[c]
                        for i, name in enumerate(out_names)})
        return res

    _CACHE["runner"] = run
    _CACHE["nc"] = nc
    return run


def _pack_x(x2d):
    """[C, T] -> [128, NTB, NCIN, 512] bf16 (8KB-contiguous partition lines)."""
    return np.ascontiguousarray(
        x2d.reshape(NCIN, 128, NTB, 512).transpose(1, 2, 0, 3)).astype(
            ml_dtypes.bfloat16)


def _pack_w(wT, nco):
    """[C_in, C_out] -> [128, nco, C_out] bf16."""
    return np.ascontiguousarray(
        wT.reshape(nco, 128, wT.shape[1]).transpose(1, 0, 2)).astype(
            ml_dtypes.bfloat16)


def _pack_w4(wT):
    """[C, G] -> [128, NCOUT, NCIN, 128] bf16 (co-major for split DMA)."""
    return np.ascontiguousarray(
        wT.reshape(NCIN, 128, NCOUT, 128).transpose(1, 2, 0, 3)).astype(
            ml_dtypes.bfloat16)


def make_in_maps(k, q, v, Wk, bk, Wq, bq, Wv, bv, Wo, bo):
    in_maps = []
    for c in range(8):
        b, g = divmod(c, 2)
        gs, ge = g * G, (g + 1) * G
        bqs = (bq[gs:ge] * SCALE).reshape(NCOUT, 128).T
        in_maps.append({
            "xq": _pack_x(q[b].T),
            "xk": _pack_x(k[b].T),
            "xv": _pack_x(v[b].T),
            "wq": _pack_w4(Wq[gs:ge, :].T * SCALE),
            "wk": _pack_w4(Wk[gs:ge, :].T),
            "wv": _pack_w(Wv[gs:ge, :].T, NCIN),
            "wo": _pack_w(Wo[:, gs:ge].T, NCOUT),
            "bq": np.ascontiguousarray(bqs, dtype=np.float32),
        })
    return in_maps


def kernel(k, q, v, Wk, bk, Wq, bq, Wv, bv, Wo, bo):
    k = np.asarray(k, dtype=np.float32)
    q = np.asarray(q, dtype=np.float32)
    v = np.asarray(v, dtype=np.float32)
    Wk, bk = np.asarray(Wk, np.float32), np.asarray(bk, np.float32)
    Wq, bq = np.asarray(Wq, np.float32), np.asarray(bq, np.float32)
    Wv, bv = np.asarray(Wv, np.float32), np.asarray(bv, np.float32)
    Wo, bo = np.asarray(Wo, np.float32), np.asarray(bo, np.float32)

    in_maps = make_in_maps(k, q, v, Wk, bk, Wq, bq, Wv, bv, Wo, bo)
    run = _get_runner()
    res = run(in_maps)
    host_bias = (bo + Wo @ bv).astype(np.float32)
    out = np.empty((B, T, C), np.float32)
    for b in range(B):
        out[b] = (res[2 * b]["out"].astype(np.float32).reshape(T, C)
                  + res[2 * b + 1]["out"].astype(np.float32).reshape(T, C)
                  + host_bias[None, :])
    return out
